# revision 8
# baseline (speedup 1.0000x reference)
"""CompressedLinear (int8 weight, per-row scale) on 8 Trainium2 NeuronCores.

Math: y[b,s,o] = sum_i x[b,s,i] * (w_int8[o,i] * scale[o]) + bias[o]

Strategy: 2(out_features) x 4(tokens) grid - a refinement of the pure
tensor-parallel hint. Sharding O eight ways gives 1376 = 10.75 tiles of
128 per core, so every (k-slice, s-chunk) pays for 11 matmul instructions
with one only 96/128 full. A 2x4 grid gives each core O_shard=5504 =
43 full 128-row tiles and a single 512-column token chunk: 43*32 = 1376
matmul instructions per core instead of 11*32*4 = 1408 (-2.3% PE time,
measured 213.5ns per instruction either way).

  - fp16 single pass: int8 weights are exact in fp16; casting x to fp16
    bounds output rel error at ~2e-4. fp8 double-pumping was measured on
    HW at exactly 2x fp16 per instruction, but exact int8 weights need a
    hi/lo fp8 pair (2 passes), so every fp8 scheme >= fp16 time.
  - x slice [4096, 512] f16 (host pre-cast/pre-transposed) stays resident
    in SBUF; loaded at startup on both HWDGE queues (evens on SP, odds on
    Activation) so the descriptor-limited 1KB-row transfers keep ahead of
    the first kt sweep.
  - W is streamed, not resident: host packs the int8 shard og-major
    ([128, kt, 512] per 4-o-tile group, 16KB contiguous rows) and SWDGE
    casts int8->f16 in flight, double-buffered one og group ahead.
  - Per-partition affine (scale, bias) fused into the PSUM eviction;
    y stores ride the Activation queue.
"""

import os
import numpy as np

import concourse.bass as bass
import concourse.tile as tile
from concourse import bacc, mybir
from concourse.bass_utils import run_bass_kernel_spmd

B = 1
S = 2048
I = 4096
O = 11008
N_CORES = 8
GRID_O = 2
GRID_S = 4
O_SHARD = O // GRID_O   # 5504
S_SHARD = S // GRID_S   # 512
P = 128
KT = I // P             # 32
OT = O_SHARD // P       # 43, all full tiles
OG_W = 4                # o-tiles per psum group
KTG = 4                 # k-slices per W stream DMA


def build_bass():
    MM_DT = mybir.dt.float16
    nc = bacc.Bacc("TRN2", target_bir_lowering=False, debug=False)

    n_og = (OT + OG_W - 1) // OG_W  # 11 groups: 10x4 + 1x3
    og_sizes = [min(OG_W, OT - g * OG_W) for g in range(n_og)]
    og_off = [0]
    for s_ in og_sizes:
        og_off.append(og_off[-1] + s_)

    xs = nc.dram_tensor("xs", [I, S_SHARD], MM_DT, kind="ExternalInput").ap()
    # W packed og-major on the host: for each og, [128, KT, og_w*128] with
    # 16KB-contiguous rows, all concatenated along the free dim.
    wp = nc.dram_tensor("wp", [P, KT * O_SHARD], mybir.dt.int8,
                        kind="ExternalInput").ap()
    scale = nc.dram_tensor("scale", [O_SHARD], mybir.dt.float32, kind="ExternalInput").ap()
    bias = nc.dram_tensor("bias", [O_SHARD], mybir.dt.float32, kind="ExternalInput").ap()
    yt = nc.dram_tensor("yt", [O_SHARD, S_SHARD], mybir.dt.float32, kind="ExternalOutput").ap()

    with tile.TileContext(nc) as tc:
        with (
            tc.tile_pool(name="wstream", bufs=2 * (KT // KTG)) as w_pool,
            tc.tile_pool(name="consts", bufs=1) as const_pool,
            tc.tile_pool(name="xres", bufs=KT) as x_pool,
            tc.tile_pool(name="outp", bufs=4) as out_pool,
            tc.tile_pool(name="psum", bufs=8, space="PSUM") as psum_pool,
        ):
            # PE warm-up: dependency-free matmuls keep the PE busy during
            # the initial DMA window, so the HAM clock gate opens (K=8/8)
            # before the first real matmul issues.
            warm_sb = const_pool.tile([P, P], MM_DT)
            nc.any.memset(warm_sb[:], 0.0)
            warm_ps = psum_pool.tile([P, P], mybir.dt.float32, name="warm_ps", tag="psum")
            N_WARM = 36
            for i in range(N_WARM):
                nc.tensor.matmul(
                    warm_ps[:], warm_sb[:], warm_sb[:],
                    start=(i == 0), stop=(i == N_WARM - 1),
                )

            # Resident x: 32 k-slice tiles, split across both HWDGE queues.
            x_res = []
            for kt in range(KT):
                xk = x_pool.tile([P, S_SHARD], MM_DT, tag="x")
                eng = nc.sync if kt % 2 == 0 else nc.scalar
                eng.dma_start(xk[:], xs[kt * P:(kt + 1) * P, :])
                x_res.append(xk)

            # per-partition scale/bias columns: [p, t] = value for o = t*128 + p
            scale_t = const_pool.tile([P, OT], mybir.dt.float32)
            bias_t = const_pool.tile([P, OT], mybir.dt.float32)
            nc.sync.dma_start(scale_t[:], scale.rearrange("(t p) -> p t", p=P))
            nc.sync.dma_start(bias_t[:], bias.rearrange("(t p) -> p t", p=P))

            # W stream: per og group, KT/KTG tiles of [128, KTG, og_w*128]
            # f16, SWDGE-cast from the packed int8 layout. Separate tiles
            # per ktg so matmuls only depend on their own slice.
            def emit_w(og):
                ogw = og_sizes[og] * P
                base = og_off[og] * KT * P
                tiles = []
                for g in range(KT // KTG):
                    wt_ = w_pool.tile([P, KTG, OG_W * P], MM_DT, tag="w")
                    src = wp[:, base + g * KTG * ogw: base + (g + 1) * KTG * ogw]
                    nc.gpsimd.dma_start(
                        wt_[:, :, :ogw],
                        src.rearrange("p (k f) -> p k f", k=KTG),
                    )
                    tiles.append(wt_)
                return tiles

            def emit_group(og, wtiles):
                o0 = og_off[og]
                ogn = og_sizes[og]
                psums = [
                    psum_pool.tile([P, S_SHARD], mybir.dt.float32,
                                   name=f"psum_{og}_{i}", tag="psum")
                    for i in range(ogn)
                ]
                for kt in range(KT):
                    wt_ = wtiles[kt // KTG]
                    ki = kt % KTG
                    for i in range(ogn):
                        nc.tensor.matmul(
                            psums[i][:], wt_[:, ki, i * P:(i + 1) * P],
                            x_res[kt][:],
                            start=(kt == 0), stop=(kt == KT - 1),
                        )
                for i in range(ogn):
                    ot = o0 + i
                    out_t = out_pool.tile([P, S_SHARD], mybir.dt.float32)
                    nc.vector.tensor_scalar(
                        out=out_t[:],
                        in0=psums[i][:],
                        scalar1=scale_t[:, ot:ot + 1],
                        scalar2=bias_t[:, ot:ot + 1],
                        op0=mybir.AluOpType.mult,
                        op1=mybir.AluOpType.add,
                    )
                    nc.scalar.dma_start(yt[ot * P:(ot + 1) * P, :], out_t[:])

            # Software-pipelined: W for og+1 is emitted before og's matmuls,
            # so its SWDGE casts stream while og computes.
            prev = emit_w(0)
            for og in range(n_og):
                nxt = emit_w(og + 1) if og + 1 < n_og else None
                emit_group(og, prev)
                prev = nxt

    nc.compile()
    return nc


_NC_CACHE = None


def _get_nc():
    global _NC_CACHE
    if _NC_CACHE is None:
        _NC_CACHE = build_bass()
    return _NC_CACHE


def _pack_w(w_shard_t):
    """[I, O_SHARD] int8 -> [128, KT*O_SHARD/P] og-major packed layout."""
    blocks = []
    n_og = (OT + OG_W - 1) // OG_W
    for og in range(n_og):
        ogw = min(OG_W, OT - og * OG_W) * P
        blk = w_shard_t[:, og * OG_W * P: og * OG_W * P + ogw]  # [I, ogw]
        blocks.append(
            blk.reshape(KT, P, ogw).transpose(1, 0, 2).reshape(P, KT * ogw)
        )
    return np.ascontiguousarray(np.concatenate(blocks, axis=1))


def run(inputs, trace=False, trace_cores=None, tmpdir=None):
    x = np.asarray(inputs["x"])
    w = np.asarray(inputs["weight_int8"])
    scale = np.asarray(inputs["scale"], dtype=np.float32)
    bias = np.asarray(inputs["bias"], dtype=np.float32)

    if w.dtype != np.int8:
        w = w.astype(np.int8)
    x2d = np.ascontiguousarray(x.reshape(S, I).astype(np.float32, copy=False))
    xtr = np.ascontiguousarray(x2d.T.astype(np.float16))  # [I, S] f16

    xs_by_si = [np.ascontiguousarray(xtr[:, si * S_SHARD:(si + 1) * S_SHARD])
                for si in range(GRID_S)]
    wp_by_oc, sc_by_oc, bi_by_oc = [], [], []
    for oc in range(GRID_O):
        sl = slice(oc * O_SHARD, (oc + 1) * O_SHARD)
        wp_by_oc.append(_pack_w(np.ascontiguousarray(w[sl, :].T)))
        sc_by_oc.append(np.ascontiguousarray(scale[sl]))
        bi_by_oc.append(np.ascontiguousarray(bias[sl]))

    in_maps = []
    for c in range(N_CORES):
        oc, si = c // GRID_S, c % GRID_S
        in_maps.append({
            "xs": xs_by_si[si],
            "wp": wp_by_oc[oc],
            "scale": sc_by_oc[oc],
            "bias": bi_by_oc[oc],
        })

    nc = _get_nc()
    kwargs = {}
    if trace:
        kwargs["trace"] = True
        if trace_cores is not None:
            kwargs["trace_cores"] = trace_cores
        if tmpdir is not None:
            kwargs["tmpdir"] = tmpdir
    res = run_bass_kernel_spmd(nc, in_maps, core_ids=list(range(N_CORES)), **kwargs)

    out = np.empty((S, O), dtype=np.float32)
    for c in range(N_CORES):
        oc, si = c // GRID_S, c % GRID_S
        out[si * S_SHARD:(si + 1) * S_SHARD,
            oc * O_SHARD:(oc + 1) * O_SHARD] = res.results[c]["yt"].T
    out = out.reshape(B, S, O)
    if trace:
        return out, res
    return out


def kernel(**inputs) -> np.ndarray:
    return run(inputs, trace=False)


# revision 11
# speedup vs baseline: 1.0648x; 1.0648x over previous
"""CompressedLinear (int8 weight, per-row scale) on 8 Trainium2 NeuronCores.

Math: y[b,s,o] = sum_i x[b,s,i] * (w_int8[o,i] * scale[o]) + bias[o]

Strategy: 2(out_features) x 4(tokens) grid - a refinement of the pure
tensor-parallel hint. Sharding O eight ways gives 1376 = 10.75 tiles of
128 per core, so every (k-slice, s-chunk) pays for 11 matmul instructions
with one only 96/128 full. A 2x4 grid gives each core O_shard=5504 =
43 full 128-row tiles and a single 512-column token chunk: 43*32 = 1376
matmul instructions per core instead of 11*32*4 = 1408 (-2.3% PE time,
measured 213.5ns per instruction either way).

  - fp16 single pass: int8 weights are exact in fp16; casting x to fp16
    bounds output rel error at ~2e-4. fp8 double-pumping was measured on
    HW at exactly 2x fp16 per instruction, but exact int8 weights need a
    hi/lo fp8 pair (2 passes), so every fp8 scheme >= fp16 time.
  - x slice [4096, 512] f16 (host pre-cast/pre-transposed) stays resident
    in SBUF; loaded at startup on both HWDGE queues (evens on SP, odds on
    Activation) so the descriptor-limited 1KB-row transfers keep ahead of
    the first kt sweep.
  - W is streamed, not resident: host packs the int8 shard og-major
    ([128, kt, 512] per 4-o-tile group, 16KB contiguous rows) and SWDGE
    casts int8->f16 in flight, double-buffered one og group ahead.
  - Per-partition affine (scale, bias) fused into the PSUM eviction;
    y stores ride the Activation queue.
"""

import os
import numpy as np

import concourse.bass as bass
import concourse.tile as tile
from concourse import bacc, mybir
from concourse.bass_utils import run_bass_kernel_spmd

B = 1
S = 2048
I = 4096
O = 11008
N_CORES = 8
GRID_O = 2
GRID_S = 4
O_SHARD = O // GRID_O   # 5504
S_SHARD = S // GRID_S   # 512
P = 128
KT = I // P             # 32
OT = O_SHARD // P       # 43, all full tiles
OG_W = 4                # o-tiles per psum group
KTG = 4                 # k-slices per W stream DMA


def build_bass():
    MM_DT = mybir.dt.float16
    nc = bacc.Bacc("TRN2", target_bir_lowering=False, debug=False)

    n_og = (OT + OG_W - 1) // OG_W  # 11 groups: 10x4 + 1x3
    og_sizes = [min(OG_W, OT - g * OG_W) for g in range(n_og)]
    og_off = [0]
    for s_ in og_sizes:
        og_off.append(og_off[-1] + s_)

    xs = nc.dram_tensor("xs", [I, S_SHARD], MM_DT, kind="ExternalInput").ap()
    # W packed og-major on the host: for each og, [128, KT, og_w*128] with
    # 16KB-contiguous rows, all concatenated along the free dim.
    wp = nc.dram_tensor("wp", [P, KT * O_SHARD], mybir.dt.int8,
                        kind="ExternalInput").ap()
    # scale/bias pre-arranged on host to [128, OT] so the load is 128
    # contiguous-row descriptors; the (t p) -> p t scatter form was 5504
    # 4-byte descriptors = a 25us DMA that head-of-line-blocked the queue.
    scale = nc.dram_tensor("scale", [P, OT], mybir.dt.float32, kind="ExternalInput").ap()
    bias = nc.dram_tensor("bias", [P, OT], mybir.dt.float32, kind="ExternalInput").ap()
    yt = nc.dram_tensor("yt", [O_SHARD, S_SHARD], mybir.dt.float32, kind="ExternalOutput").ap()

    with tile.TileContext(nc) as tc:
        with (
            tc.tile_pool(name="wstream", bufs=2 * (KT // KTG)) as w_pool,
            tc.tile_pool(name="consts", bufs=1) as const_pool,
            tc.tile_pool(name="xres", bufs=KT) as x_pool,
            tc.tile_pool(name="outp", bufs=4) as out_pool,
            tc.tile_pool(name="psum", bufs=8, space="PSUM") as psum_pool,
        ):
            # PE warm-up: dependency-free matmuls keep the PE busy during
            # the initial DMA window, so the HAM clock gate opens (K=8/8)
            # before the first real matmul issues.
            warm_sb = const_pool.tile([P, P], MM_DT)
            nc.any.memset(warm_sb[:], 0.0)
            warm_ps = psum_pool.tile([P, P], mybir.dt.float32, name="warm_ps", tag="psum")
            N_WARM = 36
            for i in range(N_WARM):
                nc.tensor.matmul(
                    warm_ps[:], warm_sb[:], warm_sb[:],
                    start=(i == 0), stop=(i == N_WARM - 1),
                )

            # Resident x: 32 k-slice tiles, split across both HWDGE queues.
            x_res = []
            for kt in range(KT):
                xk = x_pool.tile([P, S_SHARD], MM_DT, tag="x")
                eng = nc.sync if kt % 2 == 0 else nc.scalar
                eng.dma_start(xk[:], xs[kt * P:(kt + 1) * P, :])
                x_res.append(xk)

            # per-partition scale/bias columns: [p, t] = value for o = t*128 + p
            scale_t = const_pool.tile([P, OT], mybir.dt.float32)
            bias_t = const_pool.tile([P, OT], mybir.dt.float32)
            nc.sync.dma_start(scale_t[:], scale[:])
            nc.sync.dma_start(bias_t[:], bias[:])

            # W stream: per og group, KT/KTG tiles of [128, KTG, og_w*128]
            # f16, SWDGE-cast from the packed int8 layout. Separate tiles
            # per ktg so matmuls only depend on their own slice.
            def emit_w(og):
                ogw = og_sizes[og] * P
                base = og_off[og] * KT * P
                tiles = []
                for g in range(KT // KTG):
                    wt_ = w_pool.tile([P, KTG, OG_W * P], MM_DT, tag="w")
                    src = wp[:, base + g * KTG * ogw: base + (g + 1) * KTG * ogw]
                    nc.gpsimd.dma_start(
                        wt_[:, :, :ogw],
                        src.rearrange("p (k f) -> p k f", k=KTG),
                    )
                    tiles.append(wt_)
                return tiles

            def emit_group(og, wtiles):
                o0 = og_off[og]
                ogn = og_sizes[og]
                psums = [
                    psum_pool.tile([P, S_SHARD], mybir.dt.float32,
                                   name=f"psum_{og}_{i}", tag="psum")
                    for i in range(ogn)
                ]
                for kt in range(KT):
                    wt_ = wtiles[kt // KTG]
                    ki = kt % KTG
                    for i in range(ogn):
                        nc.tensor.matmul(
                            psums[i][:], wt_[:, ki, i * P:(i + 1) * P],
                            x_res[kt][:],
                            start=(kt == 0), stop=(kt == KT - 1),
                        )
                for i in range(ogn):
                    ot = o0 + i
                    out_t = out_pool.tile([P, S_SHARD], mybir.dt.float32)
                    nc.vector.tensor_scalar(
                        out=out_t[:],
                        in0=psums[i][:],
                        scalar1=scale_t[:, ot:ot + 1],
                        scalar2=bias_t[:, ot:ot + 1],
                        op0=mybir.AluOpType.mult,
                        op1=mybir.AluOpType.add,
                    )
                    nc.scalar.dma_start(yt[ot * P:(ot + 1) * P, :], out_t[:])

            # Software-pipelined: W for og+1 is emitted before og's matmuls,
            # so its SWDGE casts stream while og computes.
            prev = emit_w(0)
            for og in range(n_og):
                nxt = emit_w(og + 1) if og + 1 < n_og else None
                emit_group(og, prev)
                prev = nxt

    nc.compile()
    return nc


_NC_CACHE = None


def _get_nc():
    global _NC_CACHE
    if _NC_CACHE is None:
        _NC_CACHE = build_bass()
    return _NC_CACHE


def _pack_w(w_shard_t):
    """[I, O_SHARD] int8 -> [128, KT*O_SHARD/P] og-major packed layout."""
    blocks = []
    n_og = (OT + OG_W - 1) // OG_W
    for og in range(n_og):
        ogw = min(OG_W, OT - og * OG_W) * P
        blk = w_shard_t[:, og * OG_W * P: og * OG_W * P + ogw]  # [I, ogw]
        blocks.append(
            blk.reshape(KT, P, ogw).transpose(1, 0, 2).reshape(P, KT * ogw)
        )
    return np.ascontiguousarray(np.concatenate(blocks, axis=1))


def run(inputs, trace=False, trace_cores=None, tmpdir=None):
    x = np.asarray(inputs["x"])
    w = np.asarray(inputs["weight_int8"])
    scale = np.asarray(inputs["scale"], dtype=np.float32)
    bias = np.asarray(inputs["bias"], dtype=np.float32)

    if w.dtype != np.int8:
        w = w.astype(np.int8)
    x2d = np.ascontiguousarray(x.reshape(S, I).astype(np.float32, copy=False))
    xtr = np.ascontiguousarray(x2d.T.astype(np.float16))  # [I, S] f16

    xs_by_si = [np.ascontiguousarray(xtr[:, si * S_SHARD:(si + 1) * S_SHARD])
                for si in range(GRID_S)]
    wp_by_oc, sc_by_oc, bi_by_oc = [], [], []
    for oc in range(GRID_O):
        sl = slice(oc * O_SHARD, (oc + 1) * O_SHARD)
        wp_by_oc.append(_pack_w(np.ascontiguousarray(w[sl, :].T)))
        sc_by_oc.append(np.ascontiguousarray(scale[sl].reshape(OT, P).T))
        bi_by_oc.append(np.ascontiguousarray(bias[sl].reshape(OT, P).T))

    in_maps = []
    for c in range(N_CORES):
        oc, si = c // GRID_S, c % GRID_S
        in_maps.append({
            "xs": xs_by_si[si],
            "wp": wp_by_oc[oc],
            "scale": sc_by_oc[oc],
            "bias": bi_by_oc[oc],
        })

    nc = _get_nc()
    kwargs = {}
    if trace:
        kwargs["trace"] = True
        if trace_cores is not None:
            kwargs["trace_cores"] = trace_cores
        if tmpdir is not None:
            kwargs["tmpdir"] = tmpdir
    res = run_bass_kernel_spmd(nc, in_maps, core_ids=list(range(N_CORES)), **kwargs)

    out = np.empty((S, O), dtype=np.float32)
    for c in range(N_CORES):
        oc, si = c // GRID_S, c % GRID_S
        out[si * S_SHARD:(si + 1) * S_SHARD,
            oc * O_SHARD:(oc + 1) * O_SHARD] = res.results[c]["yt"].T
    out = out.reshape(B, S, O)
    if trace:
        return out, res
    return out


def kernel(**inputs) -> np.ndarray:
    return run(inputs, trace=False)


# revision 14
# speedup vs baseline: 1.0652x; 1.0004x over previous
"""CompressedLinear (int8 weight, per-row scale) on 8 Trainium2 NeuronCores.

Math: y[b,s,o] = sum_i x[b,s,i] * (w_int8[o,i] * scale[o]) + bias[o]

Strategy: 2(out_features) x 4(tokens) grid - a refinement of the pure
tensor-parallel hint. Sharding O eight ways gives 1376 = 10.75 tiles of
128 per core, so every (k-slice, s-chunk) pays for 11 matmul instructions
with one only 96/128 full. A 2x4 grid gives each core O_shard=5504 =
43 full 128-row tiles and a single 512-column token chunk: 43*32 = 1376
matmul instructions per core instead of 11*32*4 = 1408 (-2.3% PE time,
measured 213.5ns per instruction either way).

  - fp16 single pass: int8 weights are exact in fp16; casting x to fp16
    bounds output rel error at ~2e-4. fp8 double-pumping was measured on
    HW at exactly 2x fp16 per instruction, but exact int8 weights need a
    hi/lo fp8 pair (2 passes), so every fp8 scheme >= fp16 time.
  - x slice [4096, 512] f16 (host pre-cast/pre-transposed) stays resident
    in SBUF; loaded at startup on both HWDGE queues (evens on SP, odds on
    Activation) so the descriptor-limited 1KB-row transfers keep ahead of
    the first kt sweep.
  - W is streamed, not resident: host packs the int8 shard og-major
    ([128, kt, 512] per 4-o-tile group, 16KB contiguous rows) and SWDGE
    casts int8->f16 in flight, double-buffered one og group ahead.
  - Per-partition affine (scale, bias) fused into the PSUM eviction;
    y stores ride the Activation queue.
"""

import os
import numpy as np

import concourse.bass as bass
import concourse.tile as tile
from concourse import bacc, mybir
from concourse.bass_utils import run_bass_kernel_spmd

B = 1
S = 2048
I = 4096
O = 11008
N_CORES = 8
GRID_O = 2
GRID_S = 4
O_SHARD = O // GRID_O   # 5504
S_SHARD = S // GRID_S   # 512
P = 128
KT = I // P             # 32
OT = O_SHARD // P       # 43, all full tiles
OG_W = 4                # o-tiles per psum group
KTG = 4                 # k-slices per W stream DMA


def build_bass():
    MM_DT = mybir.dt.float16
    nc = bacc.Bacc("TRN2", target_bir_lowering=False, debug=False)

    n_og = (OT + OG_W - 1) // OG_W  # 11 groups: 10x4 + 1x3
    og_sizes = [min(OG_W, OT - g * OG_W) for g in range(n_og)]
    og_off = [0]
    for s_ in og_sizes:
        og_off.append(og_off[-1] + s_)

    xs = nc.dram_tensor("xs", [I, S_SHARD], MM_DT, kind="ExternalInput").ap()
    # W packed og-major on the host: for each og, [128, KT, og_w*128] with
    # 16KB-contiguous rows, all concatenated along the free dim.
    wp = nc.dram_tensor("wp", [P, KT * O_SHARD], mybir.dt.int8,
                        kind="ExternalInput").ap()
    # scale/bias pre-arranged on host to [128, OT] so the load is 128
    # contiguous-row descriptors; the (t p) -> p t scatter form was 5504
    # 4-byte descriptors = a 25us DMA that head-of-line-blocked the queue.
    scale = nc.dram_tensor("scale", [P, OT], mybir.dt.float32, kind="ExternalInput").ap()
    bias = nc.dram_tensor("bias", [P, OT], mybir.dt.float32, kind="ExternalInput").ap()
    yt = nc.dram_tensor("yt", [O_SHARD, S_SHARD], mybir.dt.float32, kind="ExternalOutput").ap()

    with tile.TileContext(nc) as tc:
        with (
            tc.tile_pool(name="wstream", bufs=2 * (KT // KTG)) as w_pool,
            tc.tile_pool(name="consts", bufs=1) as const_pool,
            tc.tile_pool(name="xres", bufs=KT // KTG) as x_pool,
            tc.tile_pool(name="outp", bufs=4) as out_pool,
            tc.tile_pool(name="psum", bufs=8, space="PSUM") as psum_pool,
        ):
            # PE warm-up: dependency-free matmuls keep the PE busy during
            # the initial DMA window, so the HAM clock gate opens (K=8/8)
            # before the first real matmul issues.
            warm_sb = const_pool.tile([P, P], MM_DT)
            nc.any.memset(warm_sb[:], 0.0)
            warm_ps = psum_pool.tile([P, P], mybir.dt.float32, name="warm_ps", tag="psum")
            N_WARM = 36
            for i in range(N_WARM):
                nc.tensor.matmul(
                    warm_ps[:], warm_sb[:], warm_sb[:],
                    start=(i == 0), stop=(i == N_WARM - 1),
                )

            # Resident x: 8 block loads of KTG k-slices each, alternating
            # across both HWDGE queues. One DMA (and one completion
            # semaphore) per block: per-k-slice DMAs were throttled by the
            # queues' small recycled semaphore pools to a few tiles ahead
            # of matmul consumption, leaking ~7us of startup stalls.
            x_blocks = []
            for b in range(KT // KTG):
                xb = x_pool.tile([P, KTG, S_SHARD], MM_DT, tag="x")
                eng = nc.sync if b % 2 == 0 else nc.scalar
                eng.dma_start(
                    xb[:],
                    xs[b * KTG * P:(b + 1) * KTG * P, :].rearrange(
                        "(k p) s -> p k s", p=P),
                )
                x_blocks.append(xb)
            x_res = [x_blocks[kt // KTG][:, kt % KTG, :] for kt in range(KT)]

            # per-partition scale/bias columns: [p, t] = value for o = t*128 + p
            scale_t = const_pool.tile([P, OT], mybir.dt.float32)
            bias_t = const_pool.tile([P, OT], mybir.dt.float32)
            nc.sync.dma_start(scale_t[:], scale[:])
            nc.sync.dma_start(bias_t[:], bias[:])

            # W stream: per og group, KT/KTG tiles of [128, KTG, og_w*128]
            # f16, SWDGE-cast from the packed int8 layout. Separate tiles
            # per ktg so matmuls only depend on their own slice.
            def emit_w(og):
                ogw = og_sizes[og] * P
                base = og_off[og] * KT * P
                tiles = []
                for g in range(KT // KTG):
                    wt_ = w_pool.tile([P, KTG, OG_W * P], MM_DT, tag="w")
                    src = wp[:, base + g * KTG * ogw: base + (g + 1) * KTG * ogw]
                    nc.gpsimd.dma_start(
                        wt_[:, :, :ogw],
                        src.rearrange("p (k f) -> p k f", k=KTG),
                    )
                    tiles.append(wt_)
                return tiles

            def emit_group(og, wtiles):
                o0 = og_off[og]
                ogn = og_sizes[og]
                psums = [
                    psum_pool.tile([P, S_SHARD], mybir.dt.float32,
                                   name=f"psum_{og}_{i}", tag="psum")
                    for i in range(ogn)
                ]
                for kt in range(KT):
                    wt_ = wtiles[kt // KTG]
                    ki = kt % KTG
                    for i in range(ogn):
                        nc.tensor.matmul(
                            psums[i][:], wt_[:, ki, i * P:(i + 1) * P],
                            x_res[kt],
                            start=(kt == 0), stop=(kt == KT - 1),
                        )
                for i in range(ogn):
                    ot = o0 + i
                    out_t = out_pool.tile([P, S_SHARD], mybir.dt.float32)
                    nc.vector.tensor_scalar(
                        out=out_t[:],
                        in0=psums[i][:],
                        scalar1=scale_t[:, ot:ot + 1],
                        scalar2=bias_t[:, ot:ot + 1],
                        op0=mybir.AluOpType.mult,
                        op1=mybir.AluOpType.add,
                    )
                    nc.scalar.dma_start(yt[ot * P:(ot + 1) * P, :], out_t[:])

            # Software-pipelined: W for og+1 is emitted before og's matmuls,
            # so its SWDGE casts stream while og computes.
            prev = emit_w(0)
            for og in range(n_og):
                nxt = emit_w(og + 1) if og + 1 < n_og else None
                emit_group(og, prev)
                prev = nxt

    nc.compile()
    return nc


_NC_CACHE = None


def _get_nc():
    global _NC_CACHE
    if _NC_CACHE is None:
        _NC_CACHE = build_bass()
    return _NC_CACHE


def _pack_w(w_shard_t):
    """[I, O_SHARD] int8 -> [128, KT*O_SHARD/P] og-major packed layout."""
    blocks = []
    n_og = (OT + OG_W - 1) // OG_W
    for og in range(n_og):
        ogw = min(OG_W, OT - og * OG_W) * P
        blk = w_shard_t[:, og * OG_W * P: og * OG_W * P + ogw]  # [I, ogw]
        blocks.append(
            blk.reshape(KT, P, ogw).transpose(1, 0, 2).reshape(P, KT * ogw)
        )
    return np.ascontiguousarray(np.concatenate(blocks, axis=1))


def run(inputs, trace=False, trace_cores=None, tmpdir=None):
    x = np.asarray(inputs["x"])
    w = np.asarray(inputs["weight_int8"])
    scale = np.asarray(inputs["scale"], dtype=np.float32)
    bias = np.asarray(inputs["bias"], dtype=np.float32)

    if w.dtype != np.int8:
        w = w.astype(np.int8)
    x2d = np.ascontiguousarray(x.reshape(S, I).astype(np.float32, copy=False))
    xtr = np.ascontiguousarray(x2d.T.astype(np.float16))  # [I, S] f16

    xs_by_si = [np.ascontiguousarray(xtr[:, si * S_SHARD:(si + 1) * S_SHARD])
                for si in range(GRID_S)]
    wp_by_oc, sc_by_oc, bi_by_oc = [], [], []
    for oc in range(GRID_O):
        sl = slice(oc * O_SHARD, (oc + 1) * O_SHARD)
        wp_by_oc.append(_pack_w(np.ascontiguousarray(w[sl, :].T)))
        sc_by_oc.append(np.ascontiguousarray(scale[sl].reshape(OT, P).T))
        bi_by_oc.append(np.ascontiguousarray(bias[sl].reshape(OT, P).T))

    in_maps = []
    for c in range(N_CORES):
        oc, si = c // GRID_S, c % GRID_S
        in_maps.append({
            "xs": xs_by_si[si],
            "wp": wp_by_oc[oc],
            "scale": sc_by_oc[oc],
            "bias": bi_by_oc[oc],
        })

    nc = _get_nc()
    kwargs = {}
    if trace:
        kwargs["trace"] = True
        if trace_cores is not None:
            kwargs["trace_cores"] = trace_cores
        if tmpdir is not None:
            kwargs["tmpdir"] = tmpdir
    res = run_bass_kernel_spmd(nc, in_maps, core_ids=list(range(N_CORES)), **kwargs)

    out = np.empty((S, O), dtype=np.float32)
    for c in range(N_CORES):
        oc, si = c // GRID_S, c % GRID_S
        out[si * S_SHARD:(si + 1) * S_SHARD,
            oc * O_SHARD:(oc + 1) * O_SHARD] = res.results[c]["yt"].T
    out = out.reshape(B, S, O)
    if trace:
        return out, res
    return out


def kernel(**inputs) -> np.ndarray:
    return run(inputs, trace=False)
